# revision 9
# baseline (speedup 1.0000x reference)
"""BinarizedFCLayer forward on 8 trn2 NeuronCores.

    out = X @ sign(W).T      X: [8192, 2048] f32, W: [2048, 2048] f32
                             sign(w) = +1 if w >= 0 else -1

Strategy
--------
Data-parallel over the batch dim of X: core c computes rows
[c*1024, (c+1)*1024) of the output; W is replicated.

Per core (M=1024, K=2048, N=2048 -> 512 matmuls of N=512 ~= 110.9 us PE
at the warm 2.4 GHz issue rate; 24 MiB of f32 input reads ~= 70.3 us at
the 358 GB/s per-core HBM limit):
  * TensorE contracts over the partition dim, so both operands carry K on
    partitions. The host passes X^T shards and W^T (pure layout prep).
  * X^T: SWDGE cast-DMA f32->fp16 into a resident tile (fp16 keeps 11
    mantissa bits -> output rel err ~2e-4 vs the fp32 reference).
  * W^T: SWDGE cast-DMA f32->bf16 (bf16 keeps the f32 exponent, so
    sign(bf16(w)) == sign(w)); ONE DVE pass binarizes to +-0.5 fp16
    ((w >= 0) - 0.5); the missing x2 is folded into the PSUM->SBUF
    activation copy (scale=2.0, exact power-of-two).
  * Input DMA pieces are issued as the first gpsimd instructions in an
    order chosen by an offline delivery-vs-consumption search: fine W/X
    pieces interleaved ~1:1 by bytes so unlocked matmul work stays ahead
    of the PE once it starts real work (~19 us in).
  * PE: units of one m-tile (128 rows x one 512-wide n-chunk) = 1 PSUM
    bank each, 16 k-matmuls; up to 8 units in flight. The matmul stream
    follows the same offline schedule, so mid-stream stalls stay < 2 us
    and the HAM clock gate never re-throttles. Warm-up matmuls bridge
    the DMA prologue so real matmuls start at the full 2.4 GHz rate.
  * PSUM->SBUF copies (ACT, scale=2.0) and 0.25 MiB output stores both
    ride ScalarE (its own HWDGE queue): in-engine ordering, short tail.

The walrus build here allows at most ONE sync wait per instruction, so a
post-pass splits any multi-wait instruction into single-wait NoOps on the
same engine placed immediately before it.
"""

import numpy as np

try:
    import concourse.bass as bass
except ImportError:  # harness may run from a bare directory
    import sys
    for p in ("/opt/trn_rl_repo", "/root/.axon_site/_ro/trn_rl_repo"):
        if p not in sys.path:
            sys.path.append(p)
    import concourse.bass as bass

import concourse.mybir as mybir
from concourse.tile import TileContext
from concourse.bass_utils import run_bass_kernel_spmd

P = 128
N_CORES = 8
M_FULL, K, N = 8192, 2048, 2048
M = M_FULL // N_CORES          # 1024 rows of X per core
KT = K // P                    # 16 k-tiles
MT = M // P                    # 8 m-tiles of 128
NCH, NW = 4, 512               # 4 n-chunks of 512 (one PSUM bank each)
N_WARM = 165                   # dummy matmuls bridging preamble -> first data

f32 = mybir.dt.float32
f16 = mybir.dt.float16
bf16 = mybir.dt.bfloat16

# Input DMA piece order, from an offline search over delivery schedules
# (W nn, kt_lo, kt_hi) covers wt3[:, kt_lo:kt_hi, nn*512:(nn+1)*512];
# (X mq, kt_lo, kt_hi) covers xt3[:, kt_lo:kt_hi, mq*256:(mq+1)*256).
PIECE_ORDER = [
    ('W', 1, 0, 8),
    ('W', 0, 0, 2),
    ('X', 1, 0, 8),
    ('X', 2, 0, 16),
    ('W', 0, 2, 4),
    ('W', 0, 4, 8),
    ('W', 1, 8, 16),
    ('X', 1, 8, 16),
    ('X', 3, 0, 16),
    ('X', 0, 0, 2),
    ('W', 0, 8, 16),
    ('W', 3, 0, 8),
    ('X', 0, 2, 4),
    ('W', 2, 0, 8),
    ('X', 0, 4, 8),
    ('W', 2, 8, 16),
    ('W', 3, 8, 16),
    ('X', 0, 8, 16),
]

# Delivery/consumption model used to derive the static matmul order
# (calibrated against HW traces of this kernel).
EMIT_T0 = 9.4         # us: Q7 emission start + SWDGE first-byte latency
EMIT_US = 1.0        # us of serial Q7 emission per dma_start
READ_RATE = 2.65      # us per MiB of HBM reads, no store overlap
RECEIPT = 1.2         # us from last byte to semaphore fire (HBM receipt)
BIN_PER_MIB = 0.55    # us DVE binarize per MiB(read) of W piece
BIN_FIX = 0.12
PE_T0 = 7.7           # us: PE queue starts draining
MM_US = 0.2165        # warm matmul issue period at N=512


def _piece_mib(p):
    kind, _, klo, khi = p
    return P * (khi - klo) * (512 if kind == 'W' else 256) * 4 / (1 << 20)


def _derive_exec_order():
    """Greedy consumption of the modeled delivery timeline -> static
    (nn, mq, kt) schedule; each step is a PSUM-bank-alternating pair of
    matmuls (mo=0,1) so drain always overlaps the next fill. At most 4
    pairs (8 banks) in flight."""
    avail = {}
    t = EMIT_T0
    emit_end = EMIT_T0
    for p in PIECE_ORDER:
        emit_end += EMIT_US
        start = max(t, emit_end)
        t = start + _piece_mib(p) * READ_RATE
        avail[p] = t + RECEIPT + (BIN_FIX + _piece_mib(p) * BIN_PER_MIB
                                  if p[0] == 'W' else 0.0)
    R = {}
    for nn in range(NCH):
        for mq in range(4):
            for kt in range(KT):
                w = min(ta for q, ta in avail.items()
                        if q[0] == 'W' and q[1] == nn and q[2] <= kt < q[3])
                x = min(ta for q, ta in avail.items()
                        if q[0] == 'X' and q[1] == mq and q[2] <= kt < q[3])
                R[(nn, mq, kt)] = max(w, x)
    units = [(nn, mq) for nn in range(NCH) for mq in range(4)]
    unit_order = sorted(
        units, key=lambda u: (max(R[(u[0], u[1], kt)] for kt in range(KT)),
                              R[(u[0], u[1], 0)]))
    ptr = {u: 0 for u in units}
    done = {u: False for u in units}
    open_units = []
    t = PE_T0
    order = []
    while len(order) < len(units) * KT:
        best, best_r = None, None
        for u in unit_order:
            if done[u]:
                continue
            if u not in open_units and len(open_units) >= 4:
                continue
            r = R[(u[0], u[1], ptr[u])]
            if r <= t:
                best = u
                break
            if best_r is None or r < best_r:
                best_r, best = r, u
        u = best
        t = max(t, R[(u[0], u[1], ptr[u])]) + 2 * MM_US
        if u not in open_units:
            open_units.append(u)
        order.append((u[0], u[1], ptr[u]))
        ptr[u] += 1
        if ptr[u] == KT:
            done[u] = True
            open_units.remove(u)
    return order


def _split_multiwait_instructions(nc: bass.Bass) -> int:
    """walrus codegen rejects >1 sync wait per instruction. Hoist extra waits
    onto fresh single-wait NoOps on the same engine right before the
    offending instruction (same-engine sequential waits are equivalent)."""
    n_split = 0
    for fn in nc.m.functions:
        for blk in fn.blocks:
            out = []
            for inst in blk.instructions:
                si = inst.sync_info
                if si is not None and si.on_wait and len(si.on_wait) > 1:
                    waits = list(si.on_wait)
                    for j, w in enumerate(waits[:-1]):
                        nop = mybir.InstNoOp(
                            name=f"{inst.name}_wsplit{j}", ins=[], outs=[])
                        nop.engine = inst.engine
                        nop.sync_info = mybir.SyncInfo(
                            on_wait=[w], on_update=[])
                        out.append(nop)
                        n_split += 1
                    inst.sync_info = mybir.SyncInfo(
                        on_wait=[waits[-1]],
                        on_update=list(si.on_update or []))
                out.append(inst)
            blk.instructions[:] = out
    return n_split


def _build_nc() -> bass.Bass:
    exec_order = _derive_exec_order()

    nc = bass.Bass()
    xt = nc.declare_dram_parameter("xt", [K, M], f32, isOutput=False)
    wt = nc.declare_dram_parameter("wt", [K, N], f32, isOutput=False)
    out = nc.declare_dram_parameter("out", [M, N], f32, isOutput=True)

    xt3 = xt[:].rearrange("(kt p) m -> p kt m", p=P)    # [128, 16, 1024]
    wt3 = wt[:].rearrange("(kt p) n -> p kt n", p=P)    # [128, 16, 2048]
    out3 = out[:].rearrange("(mt p) n -> p mt n", p=P)  # [128, 8, 2048]

    with TileContext(nc) as tc:
        with (
            tc.tile_pool(name="resident", bufs=1) as res_pool,
            tc.tile_pool(name="wq", bufs=4) as wq_pool,
            tc.tile_pool(name="osb", bufs=23) as o_pool,
            tc.tile_pool(name="gate", bufs=1) as g_pool,
            tc.tile_pool(name="psum", bufs=8, space="PSUM") as p_pool,
            tc.tile_pool(name="warm", bufs=1) as warm_pool,
        ):
            xq = res_pool.tile([P, KT, M], f16, tag="xq", name="xq")
            wraw = res_pool.tile([P, KT, N], bf16, tag="wraw", name="wraw")
            wqs = [wq_pool.tile([P, KT, NW], f16, tag="wq", name=f"wq{nn}")
                   for nn in range(NCH)]

            # PE warm-up first in each queue: memset leads the DVE queue
            # (so it doesn't sit behind the binarizes in DVE FIFO order),
            # dummy matmuls lead the PE queue, bridging the DMA prologue
            # and holding the HAM clock gate at 8/8 for the real stream.
            wsrc = warm_pool.tile([P, P], f16, tag="wsrc", name="wsrc")
            nc.vector.memset(wsrc[:], 0.0)
            wps = p_pool.tile([P, NW], f32, tag="ps", name="wps")
            for _ in range(N_WARM):
                nc.tensor.matmul(wps[:, :P], lhsT=wsrc[:], rhs=wsrc[:],
                                 start=True, stop=True)

            # Input pieces: SWDGE cast-DMAs lead the gpsimd queue so it
            # starts streaming immediately; each W piece is binarized on
            # DVE the moment it lands (single pass, (w >= 0) - 0.5 ->
            # +-0.5 in fp16).
            for kind, idx, klo, khi in PIECE_ORDER:
                ks = slice(klo, khi)
                if kind == 'W':
                    ns = slice(idx * NW, (idx + 1) * NW)
                    nc.gpsimd.dma_start(out=wraw[:, ks, ns],
                                        in_=wt3[:, ks, ns])
                    nc.vector.tensor_scalar(
                        wqs[idx][:, ks, :], wraw[:, ks, ns], 0.0, 0.5,
                        mybir.AluOpType.is_ge, mybir.AluOpType.subtract)
                else:
                    ms = slice(idx * 256, (idx + 1) * 256)
                    nc.gpsimd.dma_start(out=xq[:, ks, ms],
                                        in_=xt3[:, ks, ms])

            # Store gate: a tiny sync-queue DMA that reads the last
            # input piece's tile. All output stores queue behind it in
            # sync-FIFO order, so their HBM writes never contend with
            # the input read stream (which otherwise loses ~30% of
            # HBM bandwidth to them); the buffered outputs burst out
            # during the final ~55 us of pure compute.
            lk, li, lklo, lkhi = PIECE_ORDER[-1]
            gsc = g_pool.tile([1, 4], f16 if lk == 'X' else bf16,
                              tag="gate", name="gate")
            gsrc = xq if lk == 'X' else wraw
            gcol = (li + 1) * (256 if lk == 'X' else NW)
            nc.sync.dma_start(
                out=gsc[:],
                in_=gsrc[0:1, lkhi - 1:lkhi, gcol - 4:gcol])
            # Real matmuls in the scheduled order. Unit (nn, mq) = two
            # PSUM banks (mo=0,1) alternating every matmul so the PE
            # drain of one bank overlaps the fill of the other; both
            # matmuls of a step share the same rhs slice. On a unit's
            # last step each bank is copied to SBUF (ACT, scale=2.0
            # completes the binarization) and stored from the idle sync
            # HWDGE queue as a 0.25 MiB DMA.
            unit_psums = {}
            n_done = 0
            for nn, mq, kt in exec_order:
                u = (nn, mq)
                if kt == 0:
                    unit_psums[u] = [
                        p_pool.tile([P, NW], f32, tag="ps",
                                    name=f"ps{nn}_{mq}_{mo}")
                        for mo in range(2)
                    ]
                for mo in range(2):
                    mcol = mq * 256 + mo * P
                    nc.tensor.matmul(
                        unit_psums[u][mo][:],
                        lhsT=xq[:, kt, mcol:mcol + P],
                        rhs=wqs[nn][:, kt, :],
                        start=(kt == 0),
                        stop=(kt == KT - 1),
                    )
                if kt == KT - 1:
                    n_done += 1
                    for mo in range(2):
                        osb = o_pool.tile([P, NW], f32, tag="osb",
                                          name=f"osb{nn}_{mq}_{mo}")
                        if mo == 1 and n_done > 12:
                            # late units: x2 copy on DVE (long idle by now)
                            # so it runs parallel with mo0's ACT copy and
                            # the final store issues ~0.7 us sooner.
                            nc.vector.tensor_scalar(
                                osb[:], unit_psums[u][mo][:], 2.0, None,
                                mybir.AluOpType.mult)
                        else:
                            nc.scalar.activation(
                                out=osb[:], in_=unit_psums[u][mo][:],
                                func=mybir.ActivationFunctionType.Copy,
                                scale=2.0)
                        nc.sync.dma_start(
                            out=out3[:, mq * 2 + mo, nn * NW:(nn + 1) * NW],
                            in_=osb[:])

    _split_multiwait_instructions(nc)
    return nc


_NC_CACHE = None


def _get_nc() -> bass.Bass:
    global _NC_CACHE
    if _NC_CACHE is None:
        _NC_CACHE = _build_nc()
    return _NC_CACHE


def _run(inputs: dict, trace: bool = False, **kw):
    X = np.asarray(inputs["X"], dtype=np.float32)
    W = np.asarray(inputs["W"], dtype=np.float32)
    assert X.shape == (M_FULL, K) and W.shape == (N, K)

    XT = np.ascontiguousarray(X.T)            # [K, M_FULL]
    WT = np.ascontiguousarray(W.T)            # [K, N]
    in_maps = [
        {"xt": np.ascontiguousarray(XT[:, c * M:(c + 1) * M]), "wt": WT}
        for c in range(N_CORES)
    ]
    res = run_bass_kernel_spmd(
        _get_nc(), in_maps, list(range(N_CORES)), trace=trace, **kw)
    out = np.concatenate([res.results[c]["out"] for c in range(N_CORES)],
                         axis=0)
    return out, res


def kernel(X: np.ndarray, W: np.ndarray) -> np.ndarray:
    out, _ = _run({"X": X, "W": W})
    return out


# revision 11
# speedup vs baseline: 1.0443x; 1.0443x over previous
"""BinarizedFCLayer forward on 8 trn2 NeuronCores.

    out = X @ sign(W).T      X: [8192, 2048] f32, W: [2048, 2048] f32
                             sign(w) = +1 if w >= 0 else -1

Strategy
--------
Data-parallel over the batch dim of X: core c computes rows
[c*1024, (c+1)*1024) of the output; W is replicated.

Per core (M=1024, K=2048, N=2048 -> 512 matmuls of N=512 ~= 110.9 us PE
at the warm 2.4 GHz issue rate; 24 MiB of f32 input reads ~= 70.3 us at
the 358 GB/s per-core HBM limit):
  * TensorE contracts over the partition dim, so both operands carry K on
    partitions. The host passes X^T shards and W^T (pure layout prep).
  * X^T: SWDGE cast-DMA f32->fp16 into a resident tile (fp16 keeps 11
    mantissa bits -> output rel err ~2e-4 vs the fp32 reference).
  * W^T: SWDGE cast-DMA f32->bf16 (bf16 keeps the f32 exponent, so
    sign(bf16(w)) == sign(w)); ONE DVE pass binarizes to +-0.5 fp16
    ((w >= 0) - 0.5); the missing x2 is folded into the PSUM->SBUF
    activation copy (scale=2.0, exact power-of-two).
  * Input DMA pieces are issued as the first gpsimd instructions in an
    order chosen by an offline delivery-vs-consumption search: fine W/X
    pieces interleaved ~1:1 by bytes so unlocked matmul work stays ahead
    of the PE once it starts real work (~19 us in).
  * PE: units of one m-tile (128 rows x one 512-wide n-chunk) = 1 PSUM
    bank each, 16 k-matmuls; up to 8 units in flight. The matmul stream
    follows the same offline schedule, so mid-stream stalls stay < 2 us
    and the HAM clock gate never re-throttles. Warm-up matmuls bridge
    the DMA prologue so real matmuls start at the full 2.4 GHz rate.
  * PSUM->SBUF copies (ACT, scale=2.0) and 0.25 MiB output stores both
    ride ScalarE (its own HWDGE queue): in-engine ordering, short tail.

The walrus build here allows at most ONE sync wait per instruction, so a
post-pass splits any multi-wait instruction into single-wait NoOps on the
same engine placed immediately before it.
"""

import numpy as np

try:
    import concourse.bass as bass
except ImportError:  # harness may run from a bare directory
    import sys
    for p in ("/opt/trn_rl_repo", "/root/.axon_site/_ro/trn_rl_repo"):
        if p not in sys.path:
            sys.path.append(p)
    import concourse.bass as bass

import concourse.mybir as mybir
from concourse.tile import TileContext
from concourse.bass_utils import run_bass_kernel_spmd

P = 128
N_CORES = 8
M_FULL, K, N = 8192, 2048, 2048
M = M_FULL // N_CORES          # 1024 rows of X per core
KT = K // P                    # 16 k-tiles
MT = M // P                    # 8 m-tiles of 128
NCH, NW = 4, 512               # 4 n-chunks of 512 (one PSUM bank each)
N_WARM = 140                   # dummy matmuls bridging preamble -> first data

f32 = mybir.dt.float32
f16 = mybir.dt.float16
bf16 = mybir.dt.bfloat16

# Input DMA piece order, from an offline search over delivery schedules
# (W nn, kt_lo, kt_hi) covers wt3[:, kt_lo:kt_hi, nn*512:(nn+1)*512];
# (X mq, kt_lo, kt_hi) covers xt3[:, kt_lo:kt_hi, mq*256:(mq+1)*256).
PIECE_ORDER = [
    ('W', 1, 0, 8),
    ('X', 1, 0, 8),
    ('X', 3, 0, 16),
    ('W', 0, 0, 2),
    ('W', 0, 2, 4),
    ('W', 0, 4, 8),
    ('W', 0, 8, 16),
    ('X', 0, 0, 2),
    ('X', 0, 2, 4),
    ('X', 1, 8, 16),
    ('X', 2, 0, 16),
    ('X', 0, 4, 8),
    ('W', 1, 8, 16),
    ('X', 0, 8, 16),
    ('W', 2, 0, 8),
    ('W', 2, 8, 16),
    ('W', 3, 0, 8),
    ('W', 3, 8, 16),
]

# Delivery/consumption model used to derive the static matmul order
# (calibrated against HW traces of this kernel).
EMIT_T0 = 9.4         # us: Q7 emission start + SWDGE first-byte latency
EMIT_US = 1.0        # us of serial Q7 emission per dma_start
READ_RATE = 2.65      # us per MiB of HBM reads, no store overlap
RECEIPT = 1.2         # us from last byte to semaphore fire (HBM receipt)
BIN_PER_MIB = 0.55    # us DVE binarize per MiB(read) of W piece
BIN_FIX = 0.12
PE_T0 = 7.7           # us: PE queue starts draining
MM_US = 0.2165        # warm matmul issue period at N=512


def _piece_mib(p):
    kind, _, klo, khi = p
    return P * (khi - klo) * (512 if kind == 'W' else 256) * 4 / (1 << 20)


def _derive_exec_order():
    """Greedy consumption of the modeled delivery timeline -> static
    (nn, mq, kt) schedule; each step is a PSUM-bank-alternating pair of
    matmuls (mo=0,1) so drain always overlaps the next fill. At most 4
    pairs (8 banks) in flight."""
    avail = {}
    t = EMIT_T0
    emit_end = EMIT_T0
    for p in PIECE_ORDER:
        emit_end += EMIT_US
        start = max(t, emit_end)
        t = start + _piece_mib(p) * READ_RATE
        avail[p] = t + RECEIPT + (BIN_FIX + _piece_mib(p) * BIN_PER_MIB
                                  if p[0] == 'W' else 0.0)
    R = {}
    for nn in range(NCH):
        for mq in range(4):
            for kt in range(KT):
                w = min(ta for q, ta in avail.items()
                        if q[0] == 'W' and q[1] == nn and q[2] <= kt < q[3])
                x = min(ta for q, ta in avail.items()
                        if q[0] == 'X' and q[1] == mq and q[2] <= kt < q[3])
                R[(nn, mq, kt)] = max(w, x)
    units = [(nn, mq) for nn in range(NCH) for mq in range(4)]
    unit_order = sorted(
        units, key=lambda u: (max(R[(u[0], u[1], kt)] for kt in range(KT)),
                              R[(u[0], u[1], 0)]))
    ptr = {u: 0 for u in units}
    done = {u: False for u in units}
    open_units = []
    t = PE_T0
    order = []
    while len(order) < len(units) * KT:
        best, best_r = None, None
        for u in unit_order:
            if done[u]:
                continue
            if u not in open_units and len(open_units) >= 4:
                continue
            r = R[(u[0], u[1], ptr[u])]
            if r <= t:
                best = u
                break
            if best_r is None or r < best_r:
                best_r, best = r, u
        u = best
        t = max(t, R[(u[0], u[1], ptr[u])]) + 2 * MM_US
        if u not in open_units:
            open_units.append(u)
        order.append((u[0], u[1], ptr[u]))
        ptr[u] += 1
        if ptr[u] == KT:
            done[u] = True
            open_units.remove(u)
    return order


def _split_multiwait_instructions(nc: bass.Bass) -> int:
    """walrus codegen rejects >1 sync wait per instruction. Hoist extra waits
    onto fresh single-wait NoOps on the same engine right before the
    offending instruction (same-engine sequential waits are equivalent)."""
    n_split = 0
    for fn in nc.m.functions:
        for blk in fn.blocks:
            out = []
            for inst in blk.instructions:
                si = inst.sync_info
                if si is not None and si.on_wait and len(si.on_wait) > 1:
                    waits = list(si.on_wait)
                    for j, w in enumerate(waits[:-1]):
                        nop = mybir.InstNoOp(
                            name=f"{inst.name}_wsplit{j}", ins=[], outs=[])
                        nop.engine = inst.engine
                        nop.sync_info = mybir.SyncInfo(
                            on_wait=[w], on_update=[])
                        out.append(nop)
                        n_split += 1
                    inst.sync_info = mybir.SyncInfo(
                        on_wait=[waits[-1]],
                        on_update=list(si.on_update or []))
                out.append(inst)
            blk.instructions[:] = out
    return n_split


def _build_nc() -> bass.Bass:
    exec_order = _derive_exec_order()

    nc = bass.Bass()
    xt = nc.declare_dram_parameter("xt", [K, M], f32, isOutput=False)
    wt = nc.declare_dram_parameter("wt", [K, N], f32, isOutput=False)
    out = nc.declare_dram_parameter("out", [M, N], f32, isOutput=True)

    xt3 = xt[:].rearrange("(kt p) m -> p kt m", p=P)    # [128, 16, 1024]
    wt3 = wt[:].rearrange("(kt p) n -> p kt n", p=P)    # [128, 16, 2048]
    out3 = out[:].rearrange("(mt p) n -> p mt n", p=P)  # [128, 8, 2048]

    with TileContext(nc) as tc:
        with (
            tc.tile_pool(name="resident", bufs=1) as res_pool,
            tc.tile_pool(name="wq", bufs=4) as wq_pool,
            tc.tile_pool(name="osb", bufs=23) as o_pool,
            tc.tile_pool(name="gate", bufs=1) as g_pool,
            tc.tile_pool(name="psum", bufs=8, space="PSUM") as p_pool,
            tc.tile_pool(name="warm", bufs=1) as warm_pool,
        ):
            xq = res_pool.tile([P, KT, M], f16, tag="xq", name="xq")
            wraw = res_pool.tile([P, KT, N], bf16, tag="wraw", name="wraw")
            wqs = [wq_pool.tile([P, KT, NW], f16, tag="wq", name=f"wq{nn}")
                   for nn in range(NCH)]

            # PE warm-up first in each queue: memset leads the DVE queue
            # (so it doesn't sit behind the binarizes in DVE FIFO order),
            # dummy matmuls lead the PE queue, bridging the DMA prologue
            # and holding the HAM clock gate at 8/8 for the real stream.
            wsrc = warm_pool.tile([P, P], f16, tag="wsrc", name="wsrc")
            nc.vector.memset(wsrc[:], 0.0)
            wps = p_pool.tile([P, NW], f32, tag="ps", name="wps")
            for _ in range(N_WARM):
                nc.tensor.matmul(wps[:, :P], lhsT=wsrc[:], rhs=wsrc[:],
                                 start=True, stop=True)

            # Input pieces: SWDGE cast-DMAs lead the gpsimd queue so it
            # starts streaming immediately; each W piece is binarized on
            # DVE the moment it lands (single pass, (w >= 0) - 0.5 ->
            # +-0.5 in fp16).
            for kind, idx, klo, khi in PIECE_ORDER:
                ks = slice(klo, khi)
                if kind == 'W':
                    ns = slice(idx * NW, (idx + 1) * NW)
                    nc.gpsimd.dma_start(out=wraw[:, ks, ns],
                                        in_=wt3[:, ks, ns])
                    nc.vector.tensor_scalar(
                        wqs[idx][:, ks, :], wraw[:, ks, ns], 0.0, 0.5,
                        mybir.AluOpType.is_ge, mybir.AluOpType.subtract)
                else:
                    ms = slice(idx * 256, (idx + 1) * 256)
                    nc.gpsimd.dma_start(out=xq[:, ks, ms],
                                        in_=xt3[:, ks, ms])

            # Store gate: a tiny sync-queue DMA that reads the last
            # input piece's tile. All output stores queue behind it in
            # sync-FIFO order, so their HBM writes never contend with
            # the input read stream (which otherwise loses ~30% of
            # HBM bandwidth to them); the buffered outputs burst out
            # during the final ~55 us of pure compute.
            lk, li, lklo, lkhi = PIECE_ORDER[-1]
            gsc = g_pool.tile([1, 4], f16 if lk == 'X' else bf16,
                              tag="gate", name="gate")
            gsrc = xq if lk == 'X' else wraw
            gcol = (li + 1) * (256 if lk == 'X' else NW)
            gate_inst = nc.sync.dma_start(
                out=gsc[:],
                in_=gsrc[0:1, lkhi - 1:lkhi, gcol - 4:gcol])
            gate_inst.ins.bass_priority = 8_999_999
            # Real matmuls in the scheduled order. Unit (nn, mq) = two
            # PSUM banks (mo=0,1) alternating every matmul so the PE
            # drain of one bank overlaps the fill of the other; both
            # matmuls of a step share the same rhs slice. On a unit's
            # last step each bank is copied to SBUF (ACT, scale=2.0
            # completes the binarization) and stored from the idle sync
            # HWDGE queue as a 0.25 MiB DMA.
            unit_psums = {}
            n_done = 0
            for nn, mq, kt in exec_order:
                u = (nn, mq)
                if kt == 0:
                    unit_psums[u] = [
                        p_pool.tile([P, NW], f32, tag="ps",
                                    name=f"ps{nn}_{mq}_{mo}")
                        for mo in range(2)
                    ]
                for mo in range(2):
                    mcol = mq * 256 + mo * P
                    nc.tensor.matmul(
                        unit_psums[u][mo][:],
                        lhsT=xq[:, kt, mcol:mcol + P],
                        rhs=wqs[nn][:, kt, :],
                        start=(kt == 0),
                        stop=(kt == KT - 1),
                    )
                if kt == KT - 1:
                    n_done += 1
                    for mo in range(2):
                        osb = o_pool.tile([P, NW], f32, tag="osb",
                                          name=f"osb{nn}_{mq}_{mo}")
                        if mo == 1 and n_done > 12:
                            # late units: x2 copy on DVE (long idle by now)
                            # so it runs parallel with mo0's ACT copy and
                            # the final store issues ~0.7 us sooner.
                            nc.vector.tensor_scalar(
                                osb[:], unit_psums[u][mo][:], 2.0, None,
                                mybir.AluOpType.mult)
                        else:
                            nc.scalar.activation(
                                out=osb[:], in_=unit_psums[u][mo][:],
                                func=mybir.ActivationFunctionType.Copy,
                                scale=2.0)
                        st = nc.sync.dma_start(
                            out=out3[:, mq * 2 + mo, nn * NW:(nn + 1) * NW],
                            in_=osb[:])
                        st.ins.bass_priority = 9_000_000 + n_done * 2 + mo

    _split_multiwait_instructions(nc)
    return nc


_NC_CACHE = None


def _get_nc() -> bass.Bass:
    global _NC_CACHE
    if _NC_CACHE is None:
        _NC_CACHE = _build_nc()
    return _NC_CACHE


def _run(inputs: dict, trace: bool = False, **kw):
    X = np.asarray(inputs["X"], dtype=np.float32)
    W = np.asarray(inputs["W"], dtype=np.float32)
    assert X.shape == (M_FULL, K) and W.shape == (N, K)

    XT = np.ascontiguousarray(X.T)            # [K, M_FULL]
    WT = np.ascontiguousarray(W.T)            # [K, N]
    in_maps = [
        {"xt": np.ascontiguousarray(XT[:, c * M:(c + 1) * M]), "wt": WT}
        for c in range(N_CORES)
    ]
    res = run_bass_kernel_spmd(
        _get_nc(), in_maps, list(range(N_CORES)), trace=trace, **kw)
    out = np.concatenate([res.results[c]["out"] for c in range(N_CORES)],
                         axis=0)
    return out, res


def kernel(X: np.ndarray, W: np.ndarray) -> np.ndarray:
    out, _ = _run({"X": X, "W": W})
    return out
